# revision 7
# baseline (speedup 1.0000x reference)
"""Trainium2 Bass kernel for nn_DCTLayer: per-8x8-block 2D DCT-like transform.

Math: reference computes, per 8x8 block X of the 256x256 image,
    out_block[y, v] = sum_x A[v, x] * X[x, y],   where A = D @ D
(D = 8x8 DCT basis). out_block = (A @ X)^T.

Kernel strategy (per core, pure data parallel over batch):
  - Contiguous load of 4 images per DMA: tile xt[p=(G,x), f=(q=img*2+r, c)]
    where rows = 128*q + 8*G + x, cols c = 8*J + y.
  - ONE matmul per image with the constant 128x128 block-diagonal BD
    (16 copies of A^T, columns permuted so out partition = (G3G2|v3|G1G0))
    STATIONARY and the data MOVING (fp32r, 512-wide -> 1 cycle/row):
    ps[(G3G2,v,G1G0), (r,J,y)] = (A @ X_block)[v, y].
  - Two DVE 32x32 stream-transposes (per r-half) perform the
    within-8x8-block transpose ((..,v,..) partition -> (G,y) partition):
      T1: in  ps [p, y@1, J@8]   -> out s2 [p, y@1, g@8], g=(v,G1G0)
          partition (G3G2|v,G1G0) -> (G3G2|J5); s2 phys (v, G1G0, y)
      T2: in  s2 [p, v@32, h@1], h=(G1G0,y) -> out zt [p, v@1, J@8]
          partition (G3G2|J5) -> (G4,y3); zt phys (J5, v3)
    Result: zt[8*G+y, (r, 8*J+v)] = out rows -> fully contiguous stores.
  - Store 4 images per DMA with 1KB contiguous runs (full output rows).
"""

import sys

sys.path.insert(0, "/opt/trn_rl_repo")

from contextlib import ExitStack

import numpy as np

import concourse.bass as bass  # noqa: F401
import concourse.tile as tile
from concourse import bacc, mybir
from concourse.bass_utils import run_bass_kernel_spmd

P = 8
H = W = 256
B, C = 16, 64
NCORES = 8
BPC = B // NCORES  # batches per core
IMGS = BPC * C  # images (b,c planes) per core
ROWS = IMGS * H  # dram rows per core
GI = 4  # images per DMA group
NGRP = IMGS // GI

TRACE = False
LAST_RESULTS = None

_nc_cache = None


def _ensure_ntff_hook():
    """The agent image's antenv lacks axon_hooks; synthesize it so
    run_bass_kernel_spmd(trace=True) can capture NTFF profiles."""
    import types

    if "antenv.axon_hooks" in sys.modules:
        return
    try:
        sys.path.insert(0, "/root/.axon_site/trn_agent_boot")
        from trn_boot import _ntff_profile_via_ctypes

        hook = _ntff_profile_via_ctypes("/opt/axon/libaxon_pjrt.so")
    except Exception:
        hook = None
    mod = types.ModuleType("antenv.axon_hooks")
    mod._hook = hook
    mod.get_axon_ntff_profile_hook = lambda: mod._hook
    mod.set_axon_ntff_profile_hook = lambda h: setattr(mod, "_hook", h)
    sys.modules["antenv.axon_hooks"] = mod


def _stream_transpose(nc, out_ap, in_ap):
    """nc.vector.transpose but with opt=False AP lowering: the AP dim
    order IS the stream order for InstStreamTranspose, so the optimizer
    must not merge/reorder dims."""
    eng = nc.vector
    return eng.add_instruction(
        mybir.InstStreamTranspose(
            name=eng.bass.get_next_instruction_name(),
            ins=[eng.lower_ap(in_ap, opt=False)],
            outs=[eng.lower_ap(out_ap, opt=False)],
        )
    )


def _dct_kernel(tc, o, x, bd):
    nc = tc.nc
    with ExitStack() as ctx:
        xpool = ctx.enter_context(tc.tile_pool(name="xin", bufs=3))
        spool = ctx.enter_context(tc.tile_pool(name="smid", bufs=8))
        zpool = ctx.enter_context(tc.tile_pool(name="zout", bufs=3))
        cpool = ctx.enter_context(tc.tile_pool(name="const", bufs=1))
        ppool = ctx.enter_context(tc.tile_pool(name="ps", bufs=8, space="PSUM"))

        bdt = cpool.tile([128, 128], mybir.dt.float32r)
        nc.sync.dma_start(bdt[:], bd[:])

        for g in range(NGRP):
            # ---- load 4 images (8 x 128-row chunks), fully contiguous ----
            xt = xpool.tile([128, GI * 2 * W], mybir.dt.float32r)
            src = x[g * GI * H : (g + 1) * GI * H, :].rearrange(
                "(q p) c -> p q c", p=128
            )
            dst = xt[:].rearrange("p (q c) -> p q c", c=W)
            nc.sync.dma_start(dst, src)

            zt = zpool.tile([128, GI * 2 * W], mybir.dt.float32)
            for i in range(GI):
                xi = xt[:, i * 512 : (i + 1) * 512]
                # ---- one fp32r matmul: BD stationary, data moving ----
                # ps[(G4,v3), (r, J5, y3)] = (A @ X)[v, y] for block (16r+G4, J5)
                ps = ppool.tile([128, 512], mybir.dt.float32)
                nc.tensor.matmul(ps[:], bdt[:], xi, start=True, stop=True)

                # ---- per r-half: two 32x32 stream-transposes ----
                s2 = spool.tile([128, 512], mybir.dt.float32)
                for r in range(2):
                    # T1: partition (G3G2|v,G1G0) -> (G3G2|J5)
                    # in [p, y@1, J@8]; out [p, y@1, g@8], s2 phys (v,G1G0,y)
                    tin = ps[:, r * 256 : (r + 1) * 256].rearrange(
                        "p (J y) -> p y J", J=32, y=8
                    )
                    tout = s2[:, r * 256 : (r + 1) * 256].rearrange(
                        "p (g y) -> p y g", g=32, y=8
                    )
                    _stream_transpose(nc, tout, tin)
                    # T2: partition (G3G2|J5) -> (G4,y3) = out row 8G+y
                    # in [p, v@32, h@1], h=(G1G0,y); out [p, v@1, J@8]
                    tin2 = s2[:, r * 256 : (r + 1) * 256].rearrange(
                        "p (v h) -> p v h", v=8, h=32
                    )
                    tout2 = zt[
                        :, i * 512 + r * 256 : i * 512 + (r + 1) * 256
                    ].rearrange("p (J v) -> p v J", J=32, v=8)
                    _stream_transpose(nc, tout2, tin2)

            # ---- store 4 images, 1KB contiguous runs (full rows) ----
            dsto = o[g * GI * H : (g + 1) * GI * H, :].rearrange(
                "(q p) c -> p q c", p=128
            )
            srco = zt[:].rearrange("p (q c) -> p q c", c=W)
            nc.scalar.dma_start(dsto, srco)


def _build_nc():
    nc = bacc.Bacc(
        "TRN2", target_bir_lowering=False, debug=False, num_devices=NCORES
    )
    x_ap = nc.dram_tensor(
        "x", [ROWS, W], mybir.dt.float32r, kind="ExternalInput"
    ).ap()
    bd_ap = nc.dram_tensor(
        "bd", [128, 128], mybir.dt.float32r, kind="ExternalInput"
    ).ap()
    o_ap = nc.dram_tensor("o", [ROWS, W], mybir.dt.float32, kind="ExternalOutput").ap()
    with tile.TileContext(nc) as tc:
        _dct_kernel(tc, o_ap, x_ap, bd_ap)
    nc.compile()
    return nc


def _make_bd(dct_basis: np.ndarray) -> np.ndarray:
    """Block-diagonal A^T with columns permuted so the matmul's output
    partition index is (G3G2 | v2v1v0 | G1G0) instead of (G4 | v3)."""
    a = dct_basis.astype(np.float64) @ dct_basis.astype(np.float64)
    at = a.T.astype(np.float32)  # at[x, v] = A[v, x]
    bd = np.zeros((128, 128), dtype=np.float32)
    for g in range(16):
        for v in range(P):
            m = (g >> 2) * 32 + v * 4 + (g & 3)
            bd[g * P : (g + 1) * P, m] = at[:, v]
    return bd


def kernel(x: np.ndarray, dct_basis: np.ndarray) -> np.ndarray:
    global _nc_cache, LAST_RESULTS
    x = np.asarray(x, dtype=np.float32)
    dct_basis = np.asarray(dct_basis, dtype=np.float32)
    assert x.shape == (B, C, H, W)

    if _nc_cache is None:
        _nc_cache = _build_nc()
    nc = _nc_cache

    bd = _make_bd(dct_basis)
    in_maps = []
    for i in range(NCORES):
        xs = np.ascontiguousarray(x[i * BPC : (i + 1) * BPC]).reshape(ROWS, W)
        in_maps.append({"x": xs, "bd": bd})

    if TRACE:
        _ensure_ntff_hook()
    try:
        res = run_bass_kernel_spmd(
            nc, in_maps, core_ids=list(range(NCORES)), trace=TRACE
        )
    except ModuleNotFoundError:
        res = run_bass_kernel_spmd(
            nc, in_maps, core_ids=list(range(NCORES)), trace=False
        )
    LAST_RESULTS = res

    out = np.empty((B, C, H, W), dtype=np.float32)
    for i in range(NCORES):
        out[i * BPC : (i + 1) * BPC] = res.results[i]["o"].reshape(BPC, C, H, W)
    return out
